# revision 12
# baseline (speedup 1.0000x reference)
"""Multi-head self-attention (RoPE, causal) on 8 Trainium2 NeuronCores.

Problem: B=1, S=2048, D=1024, H=16 heads, d_k=64, causal, interleaved RoPE.

Sharding: tensor-parallel over heads for QKV+attention (2 heads/core),
then AllToAll to switch to sequence sharding, so the output projection
is fully local (each core computes y rows [256c:256c+256] for all 1024
output dims). Host concatenates row slices — no host arithmetic.

Device layouts (per core c, local heads h0=2c, h1=2c+1):
  xt   [1024,2048]  x^T (d on partitions) — replicated, fp32r
  qt/kt [128,2048]  Q^T/K^T rows: [h0-even(32) h0-odd(32) h1-even h1-odd]
                    (RoPE pair-permutation folded into weight slices;
                     scores are invariant to a shared permutation of d_k)
                    stored fp16 (fp16 matmul = full PE rate at any width)
  v_sb [128,16*65]  V' tiles [V(64) | ones] per k-tile fp16 (ones col ->
                    softmax sums ride the AV matmul as output row 64)
  scores: per k-tile pair, BOTH heads' score matmuls are issued
          back-to-back at tile_position (0,0)/(64,0) — the PE runs the
          two 64-contraction matmuls CONCURRENTLY in disjoint row-group
          halves of the array (~2x score throughput).
  exp on ACT -> pt fp16 -> AV: attn^T = V'^T P (fp16 operands)
  normalize: rec=1/sums, gpsimd partition_broadcast, DVE multiply -> fp16
  A2A shards attn^T [128e, 256s] fp16 -> each core gets attnT[:, S_c]
  WO: weights resident in SBUF as fp16 (no per-pass weight streaming);
      y[s,m] accumulated over 8 e-tiles, fp32 out.

Q/K/V projections stay fp32r (precision headroom); everything after the
projections runs fp16 (~2^-11 rounding, comparable to fp32r).
"""

import math
import numpy as np

import concourse.bass as bass
import concourse.mybir as mybir
import concourse.tile as tile
from concourse import bacc
from concourse.bass_utils import run_bass_kernel_spmd

F32 = mybir.dt.float32
F32R = mybir.dt.float32r
F16 = mybir.dt.float16
AF = mybir.ActivationFunctionType
ALU = mybir.AluOpType

S = 2048
D = 1024
H = 16
DK = 64
NCORES = 8
EC = D // NCORES          # 128 e-dims per core (2 heads)
SC = S // NCORES          # 256 s-rows per core after A2A
NQ = 512                  # q-chunk width
NJ = S // NQ              # 4 q-chunks
KT = S // 128             # 16 k-tiles
DT = D // 128             # 8 d-tiles
THETA = 10000.0

_PROGRAM = None

_HINTS = (mybir.EngineType.PE, mybir.EngineType.Activation,
          mybir.EngineType.DVE, mybir.EngineType.Pool,
          mybir.EngineType.SP)


def _build_program(reps=1, collective=True, loop_stages=("qk", "v", "att", "wo"),
                   stages=("qk", "v", "att", "a2a", "wo"), att_mode="full",
                   use_tp=True, look=2, ptbufs=6, staggered=False):
    nc = bacc.Bacc("TRN2", target_bir_lowering=False, debug=False,
                   num_devices=NCORES if collective else 1)

    # ---- DRAM I/O ----
    xt_d = nc.dram_tensor("xt", [D, S], F32R, kind="ExternalInput").ap()
    wqt_d = nc.dram_tensor("wqt", [D, EC], F32R, kind="ExternalInput").ap()
    wkt_d = nc.dram_tensor("wkt", [D, EC], F32R, kind="ExternalInput").ap()
    wvt2_d = nc.dram_tensor("wvt2", [D, 256], F32R, kind="ExternalInput").ap()
    wot_d = nc.dram_tensor("wot", [D, D], F16, kind="ExternalInput").ap()
    ctab_d = nc.dram_tensor("ctab", [128, S], F16, kind="ExternalInput").ap()
    stab_d = nc.dram_tensor("stab", [128, S], F16, kind="ExternalInput").ap()
    pswap_d = nc.dram_tensor("pswap", [128, 128], F16,
                             kind="ExternalInput").ap()
    msk_d = nc.dram_tensor("msk01", [128, 128], F16, kind="ExternalInput").ap()
    y_d = nc.dram_tensor("y_out", [SC, D], F32, kind="ExternalOutput").ap()

    # internal DRAM for the collective (fp16: halves the A2A bytes)
    a2a_in = nc.dram_tensor("a2a_in", [NCORES, EC, SC], F16)
    a2a_out = nc.dram_tensor("a2a_out", [NCORES, EC, SC], F16)

    with tile.TileContext(nc) as tc:
        with (
            tc.tile_pool(name="persist", bufs=1) as pp,
            tc.tile_pool(name="work", bufs=3) as wp,
            tc.tile_pool(name="pt_pool", bufs=ptbufs) as ptp,
            tc.tile_pool(name="psum", bufs=2, space="PSUM") as ps,
            tc.tile_pool(name="psum_att", bufs=3, space="PSUM") as psa,
        ):
            # ---- resident loads ----
            # order: small weights/masks first, then per-s-chunk j: trig + xt
            # chunks (j-major) so chunk-j consumers start after ~1/NJ of the
            # x DMA instead of all of it.
            wqt = pp.tile([128, DT * EC], F32R)   # [d-tile part, t*EC+e]
            wkt = pp.tile([128, DT * EC], F32R)
            wvt2 = pp.tile([128, DT * 256], F32R)
            wot16 = pp.tile([128, DT * D], F16)   # resident WO, fp16
            for t in range(DT):
                sl = slice(128 * t, 128 * (t + 1))
                nc.sync.dma_start(wqt[:, EC * t:EC * (t + 1)], wqt_d[sl, :])
                nc.sync.dma_start(wkt[:, EC * t:EC * (t + 1)], wkt_d[sl, :])
                nc.sync.dma_start(wvt2[:, 256 * t:256 * (t + 1)], wvt2_d[sl, :])
                nc.sync.dma_start(wot16[:, D * t:D * (t + 1)], wot_d[sl, :])
            ctab = pp.tile([128, S], F16)
            stab = pp.tile([128, S], F16)
            pswap = pp.tile([128, 128], F16)
            msk01 = pp.tile([128, 128], F16)
            nc.sync.dma_start(pswap[:], pswap_d[:])
            nc.sync.dma_start(msk01[:], msk_d[:])
            xt = [pp.tile([128, S], F32R, name=f"xt{t}") for t in range(DT)]
            for jc in range(NJ):
                csl_ = slice(NQ * jc, NQ * (jc + 1))
                nc.sync.dma_start(ctab[:, csl_], ctab_d[:, csl_])
                nc.sync.dma_start(stab[:, csl_], stab_d[:, csl_])
                for t in range(DT):
                    nc.sync.dma_start(
                        xt[t][:, csl_],
                        xt_d[128 * t:128 * (t + 1), csl_])

            qt = pp.tile([128, S], F16)   # RoPE'd Q^T
            kt = pp.tile([128, S], F16)   # RoPE'd K^T
            # v_sb[:, h, 65*i:65*i+64] = head h's V tile i; col 64 of each
            # 65-block is the softmax-sums "ones" column -- written once by
            # memset (never DMA'd: a 2-byte-strided DMA write would
            # read-modify-write around concurrent DVE writes).
            v_sb = pp.tile([128, 2, KT * 65], F16)
            nc.vector.memset(v_sb[:, :, 64::65], 1.0)
            attnT = pp.tile([128, S], F16)   # rows 64h:64h+64 = head h
            at = pp.tile([128, DT * SC], F16)   # at[:, 256t:] = e-tile t

            def at_load():
                src = a2a_out if collective else a2a_in
                nc.sync.dma_start(
                    at[:].rearrange("p (t s) -> p t s", t=DT),
                    src.ap().rearrange("t p s -> p t s"))

            def qk_stage():
              # Q/K/V projections interleaved per s-chunk j so each group
              # consumes only chunk-j x DMAs (pipeline compute under DMA).
              for j in range(NJ):
                  csl = slice(NQ * j, NQ * (j + 1))
                  for (wt, out_sb) in ((wqt, qt), (wkt, kt)):
                      g_ps = ps.tile([128, NQ], F32, name="g_ps", tag="mm")
                      for t in range(DT):
                          nc.tensor.matmul(
                              g_ps[:],
                              wt[:, EC * t:EC * (t + 1)],
                              xt[t][:, csl],
                              start=(t == 0), stop=(t == DT - 1))
                      # RoPE: rot = g*ctab + swap(g)*stab
                      # graw copy runs on ACT (idle in this phase); swap is
                      # an fp16 matmul; products on DVE.
                      graw = wp.tile([128, NQ], F16, name="graw")
                      nc.scalar.copy(graw[:], g_ps[:])
                      gsw_ps = ps.tile([128, NQ], F32, name="gsw_ps", tag="mm")
                      nc.tensor.matmul(gsw_ps[:], pswap[:], graw[:],
                                       start=True, stop=True)
                      a_sb = wp.tile([128, NQ], F16, name="a_sb")
                      nc.gpsimd.tensor_mul(a_sb[:], graw[:], ctab[:, csl])
                      b_sb = wp.tile([128, NQ], F16, name="b_sb")
                      nc.vector.tensor_mul(b_sb[:], gsw_ps[:], stab[:, csl])
                      nc.vector.tensor_add(out_sb[:, csl], a_sb[:], b_sb[:])
                  for st in range(4 * j, 4 * (j + 1)):
                      v_ps = ps.tile([128, NQ], F32, name="v_ps", tag="mm")
                      for t in range(DT):
                          nc.tensor.matmul(
                              v_ps[:, :256],
                              xt[t][:, 128 * st:128 * (st + 1)],
                              wvt2[:, 256 * t:256 * (t + 1)],
                              start=(t == 0), stop=(t == DT - 1))
                      # both heads' V tile in one strided copy
                      nc.vector.tensor_copy(
                          v_sb[:, :, 65 * st:65 * st + 64],
                          v_ps[:, :128].rearrange("p (h c) -> p h c", h=2))

            def v_stage():
              pass

            def att_stage():
              # ---- attention ----
              # k-tiles in pairs (2 per pair). For each pair, BOTH heads'
              # score matmuls are emitted adjacently, interleaved by head:
              # the PE overlaps the two 64-row-group streams (~2x).
              # One exp per (pair, head) -> pt fp16. Diagonal causal masks =
              # DVE 0/1 fp16 multiplies on the pt slices. AV issues LOOKP
              # pairs behind the scores so PE never waits on ACT.
              scale = 1.0 / math.sqrt(DK)
              LOOKP = max(1, look)
              for j in range(NJ):
                  av_ps = [psa.tile([65, NQ], F32, name=f"av_ps{h}",
                                    tag="av", bufs=2) for h in range(2)]
                  ndiag = 4 * j
                  npairs = (ndiag + 4) // 2
                  pend = {}
                  def emit_score(p):
                      i0 = 2 * p
                      rs = [i0 - ndiag, i0 + 1 - ndiag]
                      offs = [128 * r if r > 0 else 0 for r in rs]
                      ws = [NQ - o for o in offs]
                      cs = [0, ws[0]]
                      wtot = ws[0] + ws[1]
                      st2s = [psa.tile([128, 2 * NQ], F32, name=f"st2{h}",
                                       tag="st2", bufs=2) for h in range(2)]
                      for q in range(2):
                          for h in range(2):
                              hs = slice(64 * h, 64 * (h + 1))
                              nc.tensor.matmul(
                                  st2s[h][:, cs[q]:cs[q] + ws[q]],
                                  kt[hs, 128 * (i0 + q):128 * (i0 + q + 1)],
                                  qt[hs, NQ * j + offs[q]:NQ * (j + 1)],
                                  start=True, stop=True,
                                  tile_position=(64 * h, 0))
                      pts = []
                      for h in range(2):
                          pt = ptp.tile([128, 2 * NQ], F16, name="pt")
                          nc.scalar.activation(pt[:, :wtot], st2s[h][:, :wtot],
                                               AF.Exp, scale=scale)
                          if rs[0] >= 0:
                              for q in range(2):
                                  nc.vector.tensor_mul(
                                      pt[:, cs[q]:cs[q] + 128],
                                      pt[:, cs[q]:cs[q] + 128],
                                      msk01[:])
                          pts.append(pt)
                      pend[p] = (pts, i0, offs, ws, cs)
                  def emit_av(p):
                      pts, i0, offs, ws, cs = pend.pop(p)
                      for h in range(2):
                          for q in range(2):
                              ii = i0 + q
                              nc.tensor.matmul(
                                  av_ps[h][:, offs[q]:],
                                  v_sb[:, h, 65 * ii:65 * (ii + 1)],
                                  pts[h][:, cs[q]:cs[q] + ws[q]],
                                  start=(ii == 0), stop=(ii == ndiag + 3))
                  for k in range(npairs + LOOKP):
                      if k < npairs:
                          emit_score(k)
                      if k >= LOOKP and att_mode == "full":
                          emit_av(k - LOOKP)
                  for h in range(2):
                      if att_mode != "full":
                          continue
                      avu = wp.tile([65, NQ], F32, name="avu", tag="avu",
                                    bufs=2)
                      nc.vector.tensor_copy(avu[:], av_ps[h][:])
                      rec = wp.tile([1, NQ], F32, name="rec")
                      nc.vector.reciprocal(rec[:], avu[64:65, :])
                      bc = wp.tile([64, NQ], F32, name="bc", tag="b_sb")
                      nc.gpsimd.partition_broadcast(bc[:], rec[:])
                      nc.vector.tensor_mul(
                          attnT[64 * h:64 * (h + 1), NQ * j:NQ * (j + 1)],
                          avu[0:64, :], bc[:])

            def a2a_stage():
              # A2A staging: shard attn^T along s (fp16), one DMA
              nc.sync.dma_start(
                  a2a_in.ap().rearrange("r p s -> p r s"),
                  attnT[:].rearrange("p (r s) -> p r s", r=NCORES))

            def wo_stage(load_at=True):
              if load_at:
                  at_load()
              for n in range(2):          # m-chunks of 512
                  y_ps = [ps.tile([128, 512], F32, name=f"y_ps{sub}",
                                  tag="mm") for sub in range(SC // 128)]
                  for t in range(DT):
                      for sub in range(SC // 128):
                          nc.tensor.matmul(
                              y_ps[sub][:],
                              at[:, SC * t + 128 * sub:SC * t + 128 * (sub + 1)],
                              wot16[:, D * t + 512 * n:D * t + 512 * (n + 1)],
                              start=(t == 0), stop=(t == DT - 1))
                  y_sb = wp.tile([128, 2, 512], F32, name="y_sb")
                  for sub in range(SC // 128):
                      nc.scalar.copy(y_sb[:, sub, :], y_ps[sub][:])
                  nc.sync.dma_start(
                      y_d[:, 512 * n:512 * (n + 1)]
                          .rearrange("(a p) m -> p a m", p=128),
                      y_sb[:])

            def run(stage, key):
                if reps > 1 and key in loop_stages:
                    with tc.For_i(0, reps, 1, hint_engines=_HINTS):
                        stage()
                else:
                    stage()

            if reps > 1 and loop_stages == ("single",):
                with tc.For_i(0, reps, 1, hint_engines=_HINTS,
                              staggered_reset=staggered):
                    # at-loads first: in-loop they only depend on a2a_out
                    # (the collective runs once, outside the loop), so the
                    # WO inputs stream in under the qk/att phases.
                    at_load()
                    qk_stage()
                    att_stage()
                    a2a_stage()
                    wo_stage(load_at=False)
                stages = ()
            if "qk" in stages:
                run(qk_stage, "qk")
            if "v" in stages:
                run(v_stage, "v")
            if "att" in stages:
                run(att_stage, "att")
            if "a2a" in stages:
                a2a_stage()
            if collective and ("a2a" in stages or loop_stages == ("single",)):
                nc.gpsimd.collective_compute(
                    "AllToAll", ALU.bypass,
                    replica_groups=[list(range(NCORES))],
                    ins=[a2a_in.ap().opt()],
                    outs=[a2a_out.ap().opt()],
                )
            if "wo" in stages:
                run(wo_stage, "wo")

    nc.compile()
    return nc


def _get_program():
    global _PROGRAM
    if _PROGRAM is None:
        _PROGRAM = _build_program()
    return _PROGRAM


def _host_prep(x, token_positions, WQ, WK, WV, WO):
    x = np.asarray(x, dtype=np.float32)
    WQ = np.asarray(WQ, dtype=np.float32)
    WK = np.asarray(WK, dtype=np.float32)
    WV = np.asarray(WV, dtype=np.float32)
    WO = np.asarray(WO, dtype=np.float32)
    pos = np.asarray(token_positions).reshape(-1).astype(np.float32)

    xt = np.ascontiguousarray(x.reshape(S, D).T)            # [D, S]

    inv_freq = (1.0 / (THETA ** (np.arange(0, DK, 2, dtype=np.float32)
                                 / np.float32(DK)))).astype(np.float32)
    ang = pos[:, None] * inv_freq[None, :]                  # [S, 32] f32
    cos = np.cos(ang).astype(np.float32).T                  # [32, S]
    sin = np.sin(ang).astype(np.float32).T
    ctab = np.ascontiguousarray(np.tile(cos, (4, 1))).astype(np.float16)
    stab = np.ascontiguousarray(
        np.concatenate([-sin, sin, -sin, sin], axis=0)).astype(np.float16)

    pswap = np.zeros((128, 128), np.float16)
    for i in range(128):
        blk, o = divmod(i, 32)
        j = (blk ^ 1) * 32 + o
        pswap[j, i] = 1.0

    msk01 = (np.arange(128)[None, :] >= np.arange(128)[:, None]) \
        .astype(np.float16)                                 # keep f >= p

    perm = np.concatenate([np.arange(0, DK, 2), np.arange(1, DK, 2)])
    in_maps = []
    for c in range(NCORES):
        rows = np.concatenate([128 * c + 64 * l + perm for l in range(2)])
        wqt = np.ascontiguousarray(WQ[rows, :].T)           # [D, EC]
        wkt = np.ascontiguousarray(WK[rows, :].T)
        vrows = np.arange(128 * c, 128 * (c + 1))
        wvt = WV[vrows, :].T                                # [D, EC]
        wvt2 = np.ascontiguousarray(np.concatenate([wvt, wvt], axis=1))
        in_maps.append({
            "xt": xt, "wqt": wqt, "wkt": wkt, "wvt2": wvt2,
            "wot": np.ascontiguousarray(WO.T).astype(np.float16),
            "ctab": ctab, "stab": stab, "pswap": pswap,
            "msk01": msk01,
        })
    return in_maps


def kernel(x, token_positions, WQ, WK, WV, WO):
    in_maps = _host_prep(x, token_positions, WQ, WK, WV, WO)
    nc = _get_program()
    res = run_bass_kernel_spmd(nc, in_maps, list(range(NCORES)))
    y = np.concatenate([res.results[c]["y_out"] for c in range(NCORES)],
                       axis=0)
    return y.reshape(1, S, D).astype(np.float32)


# revision 30
# speedup vs baseline: 4.4366x; 4.4366x over previous
"""Multi-head self-attention (RoPE, causal) on 8 Trainium2 NeuronCores.

Problem: B=1, S=2048, D=1024, H=16 heads, d_k=64, causal, interleaved RoPE.

Sharding: tensor-parallel over heads for QKV+attention (2 heads/core),
then AllToAll to switch to sequence sharding, so the output projection
is fully local (each core computes y rows [256c:256c+256] for all 1024
output dims). Host concatenates row slices — no host arithmetic.

Device layouts (per core c, local heads h0=2c, h1=2c+1):
  xt   [1024,2048]  x^T (d on partitions) — replicated, fp32r
  qt/kt [128,2048]  Q^T/K^T rows: [h0-even(32) h0-odd(32) h1-even h1-odd]
                    (RoPE pair-permutation folded into weight slices;
                     scores are invariant to a shared permutation of d_k)
                    stored fp16 (fp16 matmul = full PE rate at any width)
  v_sb [128,2,16*65] per head: V' tiles [V(64) | ones] per k-tile, fp16
                    (ones col -> softmax sums ride the AV matmul as
                     output row 64; set once by memset, never DMA'd)
  scores: per k-tile pair, BOTH heads' score matmuls are issued
          back-to-back at tile_position (0,0)/(64,0) — the PE runs the
          two 64-contraction matmuls CONCURRENTLY in disjoint row-group
          halves of the array (~2x score throughput).
  exp on ACT -> pt fp16 -> AV: attn^T = V'^T P (fp16 operands)
  normalize: rec=1/sums, gpsimd partition_broadcast, DVE multiply -> fp16
  A2A shards attn^T [128e, 256s] fp16 -> each core gets attnT[:, S_c]
  WO: weights resident in SBUF as fp16 (no per-pass weight streaming);
      y[s,m] accumulated over 8 e-tiles, fp32 out.

Q/K/V projections and RoPE stay fp32r/f32 (precision headroom);
attention operands and the WO path run fp16 (~2^-11 rounding).
"""

import math
import numpy as np

import concourse.bass as bass
import concourse.mybir as mybir
import concourse.tile as tile
from concourse import bacc
from concourse.bass_utils import run_bass_kernel_spmd

F32 = mybir.dt.float32
F32R = mybir.dt.float32r
F16 = mybir.dt.float16
AF = mybir.ActivationFunctionType
ALU = mybir.AluOpType

S = 2048
D = 1024
H = 16
DK = 64
NCORES = 8
EC = D // NCORES          # 128 e-dims per core (2 heads)
SC = S // NCORES          # 256 s-rows per core after A2A
NQ = 512                  # q-chunk width
NJ = S // NQ              # 4 q-chunks
KT = S // 128             # 16 k-tiles
DT = D // 128             # 8 d-tiles
THETA = 10000.0

_PROGRAM = None

_HINTS = (mybir.EngineType.PE, mybir.EngineType.Activation,
          mybir.EngineType.DVE, mybir.EngineType.Pool,
          mybir.EngineType.SP)


def _build_program(reps=1, collective=True, loop_stages=("qk", "v", "att", "wo"),
                   stages=("qk", "v", "att", "a2a", "wo"), att_mode="full",
                   use_tp=True, look=3, ptbufs=8, staggered=False,
                   att16=True, av16=None, reorder=False):
    nc = bacc.Bacc("TRN2", target_bir_lowering=False, debug=False,
                   num_devices=NCORES if collective else 1)

    if av16 is None:
        av16 = att16
    FA = F16 if att16 else F32R          # score operand dtype (qt/kt)
    FP = F16 if av16 else F32R           # AV operand dtype (pt/v_sb)

    # ---- DRAM I/O ----
    xt_d = nc.dram_tensor("xt", [D, S], F32R, kind="ExternalInput").ap()
    wqt_d = nc.dram_tensor("wqt", [D, EC], F32R, kind="ExternalInput").ap()
    wkt_d = nc.dram_tensor("wkt", [D, EC], F32R, kind="ExternalInput").ap()
    wvt2_d = nc.dram_tensor("wvt2", [D, 256], F32R, kind="ExternalInput").ap()
    wot_d = nc.dram_tensor("wot", [D, D], F16, kind="ExternalInput").ap()
    ctab_d = nc.dram_tensor("ctab", [128, S], F32, kind="ExternalInput").ap()
    stab_d = nc.dram_tensor("stab", [128, S], F32, kind="ExternalInput").ap()
    pswap_d = nc.dram_tensor("pswap", [128, 128], F32R,
                             kind="ExternalInput").ap()
    msk_d = nc.dram_tensor("msk01", [128, 128], F32, kind="ExternalInput").ap()
    y_d = nc.dram_tensor("y_out", [SC, D], F32, kind="ExternalOutput").ap()

    # internal DRAM for the collective (fp16: halves the A2A bytes)
    a2a_in = nc.dram_tensor("a2a_in", [NCORES, EC, SC], F16)
    a2a_out = nc.dram_tensor("a2a_out", [NCORES, EC, SC], F16)

    with tile.TileContext(nc) as tc:
        with (
            tc.tile_pool(name="persist", bufs=1) as pp,
            tc.tile_pool(name="work", bufs=3) as wp,
            tc.tile_pool(name="pt_pool", bufs=ptbufs) as ptp,
            tc.tile_pool(name="psum", bufs=2, space="PSUM") as ps,
            tc.tile_pool(name="psum_att", bufs=3, space="PSUM") as psa,
        ):
            # ---- resident loads ----
            wqt = pp.tile([128, DT * EC], F32R)   # [d-tile part, t*EC+e]
            wkt = pp.tile([128, DT * EC], F32R)
            wvt2 = pp.tile([128, DT * 256], F32R)
            wot16 = pp.tile([128, DT * D], F16)   # resident WO, fp16
            for t in range(DT):
                sl = slice(128 * t, 128 * (t + 1))
                nc.sync.dma_start(wqt[:, EC * t:EC * (t + 1)], wqt_d[sl, :])
                nc.sync.dma_start(wkt[:, EC * t:EC * (t + 1)], wkt_d[sl, :])
                nc.sync.dma_start(wvt2[:, 256 * t:256 * (t + 1)], wvt2_d[sl, :])
                nc.sync.dma_start(wot16[:, D * t:D * (t + 1)], wot_d[sl, :])
            ctab = pp.tile([128, S], F32)
            stab = pp.tile([128, S], F32)
            pswap = pp.tile([128, 128], F32R)
            msk01 = pp.tile([128, 128], F32)
            nc.sync.dma_start(pswap[:], pswap_d[:])
            nc.sync.dma_start(msk01[:], msk_d[:])
            mska = pp.tile([128, 128], FP)        # mask in AV dtype
            if av16:
                nc.vector.tensor_copy(mska[:], msk01[:])
            xt = [pp.tile([128, S], F32R, name=f"xt{t}") for t in range(DT)]
            for jc in range(NJ):
                csl_ = slice(NQ * jc, NQ * (jc + 1))
                nc.sync.dma_start(ctab[:, csl_], ctab_d[:, csl_])
                nc.sync.dma_start(stab[:, csl_], stab_d[:, csl_])
                for t in range(DT):
                    nc.sync.dma_start(
                        xt[t][:, csl_],
                        xt_d[128 * t:128 * (t + 1), csl_])

            qt = pp.tile([128, S], FA)   # RoPE'd Q^T
            kt = pp.tile([128, S], FA)   # RoPE'd K^T
            # v_sb[:, h, 65*i:65*i+64] = head h's V tile i; col 64 of each
            # 65-block is the softmax-sums "ones" column -- written once by
            # memset (never DMA'd: a 2-byte-strided DMA write would
            # read-modify-write around concurrent DVE writes).
            v_sb = pp.tile([128, 2, KT * 65], FP)
            ones_ap = v_sb[:, :, 64::65]
            nc.vector.memset(ones_ap.bitcast(F32) if FP == F32R else ones_ap,
                             1.0)
            attnT = pp.tile([128, S], F16)   # rows 64h:64h+64 = head h
            at = pp.tile([128, DT * SC], F16)   # at[:, 256t:] = e-tile t

            def at_load():
                # one DMA per e-tile: spreads across DMA queues/engines
                src = a2a_out if collective else a2a_in
                for t in range(DT):
                    nc.sync.dma_start(at[:, SC * t:SC * (t + 1)],
                                      src.ap()[t, :, :])

            def qk_stage():
              # Q/K/V projections interleaved per s-chunk j so each group
              # consumes only chunk-j x DMAs (pipeline compute under DMA).
              for j in range(NJ):
                  csl = slice(NQ * j, NQ * (j + 1))
                  # Q and K projections first; the RoPE swap matmuls are
                  # deferred until after the first V groups so the PE does
                  # not wait on the DVE graw copies (PE<->DVE ping-pong).
                  rope_tail = []
                  for (wt, out_sb) in ((wqt, qt), (wkt, kt)):
                      g_ps = ps.tile([128, NQ], F32, name="g_ps", tag="mm")
                      for t in range(DT):
                          nc.tensor.matmul(
                              g_ps[:],
                              wt[:, EC * t:EC * (t + 1)],
                              xt[t][:, csl],
                              start=(t == 0), stop=(t == DT - 1))
                      graw = wp.tile([128, NQ], F32R, name="graw")
                      nc.vector.tensor_copy(graw[:], g_ps[:])
                      a_sb = wp.tile([128, NQ], F32, name="a_sb")
                      nc.gpsimd.tensor_mul(a_sb[:], graw[:].bitcast(F32),
                                           ctab[:, csl])
                      rope_tail.append((graw, a_sb, out_sb))
                  def rope_finish():
                      # swap matmul + combine for both Q and K
                      for graw, a_sb, out_sb in rope_tail:
                          gsw_ps = ps.tile([128, NQ], F32, name="gsw_ps",
                                           tag="mm")
                          nc.tensor.matmul(gsw_ps[:], pswap[:], graw[:],
                                           start=True, stop=True)
                          b_sb = wp.tile([128, NQ], F32, name="b_sb")
                          nc.vector.tensor_mul(b_sb[:], gsw_ps[:],
                                               stab[:, csl])
                          nc.vector.tensor_add(out_sb[:, csl], a_sb[:],
                                               b_sb[:])
                      rope_tail.clear()
                  for sti, st in enumerate(range(4 * j, 4 * (j + 1))):
                      v_ps = ps.tile([128, NQ], F32, name="v_ps", tag="mm")
                      for t in range(DT):
                          nc.tensor.matmul(
                              v_ps[:, :256],
                              xt[t][:, 128 * st:128 * (st + 1)],
                              wvt2[:, 256 * t:256 * (t + 1)],
                              start=(t == 0), stop=(t == DT - 1))
                      # both heads' V tile in one strided copy
                      nc.vector.tensor_copy(
                          v_sb[:, :, 65 * st:65 * st + 64],
                          v_ps[:, :128].rearrange("p (h c) -> p h c", h=2))
                      if sti == 1:
                          rope_finish()
                  if rope_tail:
                      rope_finish()

            def v_stage():
              pass

            def att_stage():
              # ---- attention ----
              # k-tiles in pairs (2 per pair). With reorder=True, both
              # heads' score matmuls are emitted adjacently interleaved by
              # head: the PE overlaps the two 64-row-group streams (~2x).
              # One exp per (pair, head) -> pt. Diagonal causal masks =
              # DVE 0/1 multiplies on the pt slices. AV issues LOOKP
              # pairs behind the scores so PE never waits on ACT.
              # Per k-tile (not pairs): both heads' score matmuls write one
              # [128, 2, NQ] PSUM tile (2 banks: bank h = head h), then ONE
              # exp covers both heads -> pt [128, 2, NQ] fp16. With 2-bank
              # tiles at bufs=2 (4 banks total), scores(i+1) only wait for
              # exp(i-1): PE and ACT pipeline at k-tile granularity instead
              # of strictly alternating (the old pair-tile layout serialized
              # PE scores behind every exp).
              scale = 1.0 / math.sqrt(DK)
              LOOKI = max(1, look)
              for j in range(NJ):
                  av_ps = [psa.tile([65, NQ], F32, name=f"av_ps{h}",
                                    tag="av", bufs=2) for h in range(2)]
                  ndiag = 4 * j
                  nkt = ndiag + 4
                  pend = {}
                  def emit_score(i):
                      r = i - ndiag
                      off = 128 * r if r > 0 else 0
                      w = NQ - off
                      st = psa.tile([128, 2, NQ], F32, name="st",
                                    tag="st2", bufs=2)
                      for h in range(2):
                          hs = slice(64 * h, 64 * (h + 1))
                          nc.tensor.matmul(
                              st[:, h, :w],
                              kt[hs, 128 * i:128 * (i + 1)],
                              qt[hs, NQ * j + off:NQ * (j + 1)],
                              start=True, stop=True,
                              tile_position=(64 * h, 0))
                      pt = ptp.tile([128, 2, NQ], FP, name="pt")
                      nc.scalar.activation(pt[:, :, :w], st[:, :, :w],
                                           AF.Exp, scale=scale)
                      if r >= 0:
                          for h in range(2):
                              psl = pt[:, h, 0:128]
                              if av16:
                                  nc.vector.tensor_mul(psl, psl, mska[:])
                              else:
                                  nc.vector.tensor_mul(
                                      psl, psl.bitcast(F32), msk01[:])
                      pend[i] = (pt, off, w)
                  def emit_av(i):
                      pt, off, w = pend.pop(i)
                      for h in range(2):
                          nc.tensor.matmul(
                              av_ps[h][:, off:],
                              v_sb[:, h, 65 * i:65 * (i + 1)],
                              pt[:, h, :w],
                              start=(i == 0), stop=(i == nkt - 1))
                  # AV before score within each step: when scores(i) stall
                  # on the st buffer (waiting for exp(i-2)), the ready AV
                  # matmuls must not be queued behind them in the PE FIFO.
                  for k in range(nkt + LOOKI):
                      if k >= LOOKI and att_mode == "full":
                          emit_av(k - LOOKI)
                      if k < nkt:
                          emit_score(k)
                  for h in range(2):
                      if att_mode != "full":
                          continue
                      avu = wp.tile([65, NQ], F32, name="avu", tag="avu",
                                    bufs=2)
                      nc.vector.tensor_copy(avu[:], av_ps[h][:])
                      rec = wp.tile([1, NQ], F32, name="rec")
                      nc.vector.reciprocal(rec[:], avu[64:65, :])
                      bc = wp.tile([64, NQ], F32, name="bc", tag="b_sb")
                      nc.gpsimd.partition_broadcast(bc[:], rec[:])
                      nc.vector.tensor_mul(
                          attnT[64 * h:64 * (h + 1), NQ * j:NQ * (j + 1)],
                          avu[0:64, :], bc[:])

            def a2a_stage():
              # A2A staging: shard attn^T along s (fp16), one DMA per rank
              for r in range(NCORES):
                  nc.sync.dma_start(a2a_in.ap()[r, :, :],
                                    attnT[:, SC * r:SC * (r + 1)])

            def wo_stage(load_at=True):
              if load_at:
                  at_load()
              for n in range(2):          # m-chunks of 512
                  y_ps = [ps.tile([128, 512], F32, name=f"y_ps{sub}",
                                  tag="mm") for sub in range(SC // 128)]
                  for t in range(DT):
                      for sub in range(SC // 128):
                          nc.tensor.matmul(
                              y_ps[sub][:],
                              at[:, SC * t + 128 * sub:SC * t + 128 * (sub + 1)],
                              wot16[:, D * t + 512 * n:D * t + 512 * (n + 1)],
                              start=(t == 0), stop=(t == DT - 1))
                  for sub in range(SC // 128):
                      y_sb = wp.tile([128, 512], F32, name="y_sb")
                      nc.scalar.copy(y_sb[:], y_ps[sub][:])
                      nc.sync.dma_start(
                          y_d[128 * sub:128 * (sub + 1),
                              512 * n:512 * (n + 1)], y_sb[:])

            def run(stage, key):
                if reps > 1 and key in loop_stages:
                    with tc.For_i(0, reps, 1, hint_engines=_HINTS):
                        stage()
                else:
                    stage()

            if reps > 1 and loop_stages == ("single",):
                with tc.For_i(0, reps, 1, hint_engines=_HINTS,
                              staggered_reset=staggered):
                    # at-loads first: in-loop they only depend on a2a_out
                    # (the collective runs once, outside the loop), so the
                    # WO inputs stream in under the qk/att phases.
                    at_load()
                    qk_stage()
                    att_stage()
                    a2a_stage()
                    wo_stage(load_at=False)
                stages = ()
            if "qk" in stages:
                run(qk_stage, "qk")
            if "v" in stages:
                run(v_stage, "v")
            if "att" in stages:
                run(att_stage, "att")
            if "a2a" in stages:
                a2a_stage()
            if collective and ("a2a" in stages or loop_stages == ("single",)):
                nc.gpsimd.collective_compute(
                    "AllToAll", ALU.bypass,
                    replica_groups=[list(range(NCORES))],
                    ins=[a2a_in.ap().opt()],
                    outs=[a2a_out.ap().opt()],
                )
            if "wo" in stages:
                run(wo_stage, "wo")

    nc.compile()
    return nc


def _get_program():
    global _PROGRAM
    if _PROGRAM is None:
        _PROGRAM = _build_program()
    return _PROGRAM


def _host_prep(x, token_positions, WQ, WK, WV, WO):
    x = np.asarray(x, dtype=np.float32)
    WQ = np.asarray(WQ, dtype=np.float32)
    WK = np.asarray(WK, dtype=np.float32)
    WV = np.asarray(WV, dtype=np.float32)
    WO = np.asarray(WO, dtype=np.float32)
    pos = np.asarray(token_positions).reshape(-1).astype(np.float32)

    xt = np.ascontiguousarray(x.reshape(S, D).T)            # [D, S]

    inv_freq = (1.0 / (THETA ** (np.arange(0, DK, 2, dtype=np.float32)
                                 / np.float32(DK)))).astype(np.float32)
    ang = pos[:, None] * inv_freq[None, :]                  # [S, 32] f32
    cos = np.cos(ang).astype(np.float32).T                  # [32, S]
    sin = np.sin(ang).astype(np.float32).T
    ctab = np.ascontiguousarray(np.tile(cos, (4, 1)))       # [128, S]
    stab = np.ascontiguousarray(
        np.concatenate([-sin, sin, -sin, sin], axis=0))     # [128, S]

    pswap = np.zeros((128, 128), np.float32)
    for i in range(128):
        blk, o = divmod(i, 32)
        j = (blk ^ 1) * 32 + o
        pswap[j, i] = 1.0

    msk01 = (np.arange(128)[None, :] >= np.arange(128)[:, None]) \
        .astype(np.float32)                                 # keep f >= p

    perm = np.concatenate([np.arange(0, DK, 2), np.arange(1, DK, 2)])
    in_maps = []
    for c in range(NCORES):
        rows = np.concatenate([128 * c + 64 * l + perm for l in range(2)])
        wqt = np.ascontiguousarray(WQ[rows, :].T)           # [D, EC]
        wkt = np.ascontiguousarray(WK[rows, :].T)
        vrows = np.arange(128 * c, 128 * (c + 1))
        wvt = WV[vrows, :].T                                # [D, EC]
        wvt2 = np.ascontiguousarray(np.concatenate([wvt, wvt], axis=1))
        in_maps.append({
            "xt": xt, "wqt": wqt, "wkt": wkt, "wvt2": wvt2,
            "wot": np.ascontiguousarray(WO.T).astype(np.float16),
            "ctab": ctab, "stab": stab, "pswap": pswap,
            "msk01": msk01,
        })
    return in_maps


def kernel(x, token_positions, WQ, WK, WV, WO):
    in_maps = _host_prep(x, token_positions, WQ, WK, WV, WO)
    nc = _get_program()
    res = run_bass_kernel_spmd(nc, in_maps, list(range(NCORES)))
    y = np.concatenate([res.results[c]["y_out"] for c in range(NCORES)],
                       axis=0)
    return y.reshape(1, S, D).astype(np.float32)
